# revision 10
# baseline (speedup 1.0000x reference)
"""Fused SwiGLU MLP (gate/up/down) Trainium2 Bass kernel.

Problem: y = down( silu(x @ Wg^T) * (x @ Wu^T) ) with
  x  [B=2, S=2048, H=4096]  f32
  Wg [I=11008, H]           f32   (gate proj, [out,in])
  Wu [I=11008, H]           f32
  Wd [H, I]                 f32

Strategy: data-parallel over tokens across the 8 NeuronCores.
Each core gets T = 4096/8 = 512 tokens and the full (replicated) weights,
computing the entire MLP for its token shard.  No collectives; the host
just concatenates the 8 token shards.

Per-core work is 70.9 G MAC = 8256 matmuls of 128x128x512.  All matmul
operands are bf16 (1 PE cycle/row at 2.4 GHz, same rate as f32r, but half
the HBM traffic: ~270 MB/core vs 570 MB f32, so DMA stays well clear of
the PE roofline of ~1.76 ms).  f32 PSUM accumulation keeps the rel error
~4e-3.

Two-pass structure per core (v2; the v1 chunked structure interleaved
gate/up and down per i-chunk, costing PSUM-bank contention + a DVE
accumulate of y per chunk):
  pass 1: for each of 86 i-subtiles: psg/psu[128i, T] accumulate over
          32 h-subtiles; hm[:, isub, :] = silu(psg) * psu  (bf16, resident:
          full [128, 86, 512] = 86 KB/partition in SBUF)
  pass 2: for each o-chunk (8 x 512) hold 4 PSUM banks py[128t, 512o]
          (one per 128-token tile) and accumulate over all 86 i-subtiles;
          wd tile loaded once per (osc, isub), reused for 4 matmuls.
          Drain via ACT copy -> DMA out.  No DVE adds, no y memset.

All transposes/tiling are done on HOST in numpy so every device DMA is a
plain contiguous partition-major copy:
  x_host  [HS=32, 128, T]      x^T tiled: [hs, p, t] = x[t, hs*128+p]
  wg_host [NI=86, 128, HS, 128]  [isub, p, hs, ii] = Wg[isub*128+ii, hs*128+p]
  wu_host same
  wd_host [NO=8, NI, 128, 512]   [osc, isub, p, oo] = Wd[osc*512+oo, isub*128+p]
  y out   [TT=4, 128, H]       y[tt*128+p, o]  f32
"""

import numpy as np
import ml_dtypes

import concourse.bass as bass
import concourse.mybir as mybir
import concourse.tile as tile
from concourse import bacc
from concourse.bass_utils import run_bass_kernel_spmd

F32 = mybir.dt.float32
BF16 = mybir.dt.bfloat16
BF16_NP = ml_dtypes.bfloat16
P = 128
OCW = 512  # o-chunk width (one PSUM bank of f32)

# full-size problem constants
B, S, H, I = 2, 2048, 4096, 11008
NCORES = 8
T = (B * S) // NCORES  # 512 tokens per core


def build_nc(T, H, I, wg_bufs=3, wd_bufs=16, use_silu=True):
    assert T % P == 0 and T <= 512
    assert H % OCW == 0 and I % P == 0
    HS = H // P   # h subtiles (contraction for gate/up)
    NI = I // P   # i subtiles (with I padded to a multiple of 128 by host)
    NO = H // OCW
    TT = T // P

    nc = bacc.Bacc("TRN2", target_bir_lowering=False, debug=False)
    x_d = nc.dram_tensor("x", [HS, P, T], BF16, kind="ExternalInput").ap()
    wg_d = nc.dram_tensor("wg", [NI, P, HS, P], BF16, kind="ExternalInput").ap()
    wu_d = nc.dram_tensor("wu", [NI, P, HS, P], BF16, kind="ExternalInput").ap()
    wd_d = nc.dram_tensor("wd", [NO, NI, P, OCW], BF16, kind="ExternalInput").ap()
    y_d = nc.dram_tensor("y", [TT, P, H], F32, kind="ExternalOutput").ap()

    with tile.TileContext(nc) as tc:
        with (
            tc.tile_pool(name="xp", bufs=1) as xp,
            tc.tile_pool(name="hmp", bufs=1) as hmp,
            tc.tile_pool(name="wgp", bufs=wg_bufs) as wgp,
            tc.tile_pool(name="wup", bufs=wg_bufs) as wup,
            tc.tile_pool(name="wdp", bufs=wd_bufs) as wdp,
            tc.tile_pool(name="sgp", bufs=2) as sgp,
            tc.tile_pool(name="yop", bufs=4) as yop,
            tc.tile_pool(name="ps", bufs=8, space="PSUM") as ps,
        ):
            # DMA issue order matters: queues are FIFO, so stream the first
            # i-subtile's weights and x in exact consumption order (per
            # 8-h-subtile octet), then subtile 1's weights, so the PE starts
            # after ~0.4 MiB instead of waiting behind 4 MiB of x.  Tile
            # tracks slice-level deps, so partial-tile DMAs unblock the
            # matmuls that only read those slices.
            xt = xp.tile([P, HS, T], BF16, name="xt")
            gt0 = wgp.tile([P, HS, P], BF16, tag="wg", name="gt0")
            ut0 = wup.tile([P, HS, P], BF16, tag="wu", name="ut0")
            for c in range(0, HS, 8):
                nc.sync.dma_start(out=gt0[:, c:c + 8, :], in_=wg_d[0, :, c:c + 8, :])
                nc.sync.dma_start(out=ut0[:, c:c + 8, :], in_=wu_d[0, :, c:c + 8, :])
                for hs in range(c, c + 8):
                    nc.sync.dma_start(out=xt[:, hs, :], in_=x_d[hs])
            gt1 = wgp.tile([P, HS, P], BF16, tag="wg", name="gt1")
            nc.sync.dma_start(out=gt1, in_=wg_d[1])
            ut1 = wup.tile([P, HS, P], BF16, tag="wu", name="ut1")
            nc.sync.dma_start(out=ut1, in_=wu_d[1])
            head = [(gt0, ut0), (gt1, ut1)]
            # resident h_mid, [128i, isub, t] bf16 (86 KB/partition)
            hm = hmp.tile([P, NI, T], BF16)

            # ---- pass 1: gate/up projections + silu*up ----
            for isub in range(NI):
                if isub < 2:
                    gt, ut = head[isub]
                else:
                    gt = wgp.tile([P, HS, P], BF16, tag="wg")
                    nc.sync.dma_start(out=gt, in_=wg_d[isub])
                    ut = wup.tile([P, HS, P], BF16, tag="wu")
                    nc.sync.dma_start(out=ut, in_=wu_d[isub])
                psg = ps.tile([P, T], F32, tag="ps", name="psg")
                psu = ps.tile([P, T], F32, tag="ps", name="psu")
                for hs in range(HS):
                    first, last = hs == 0, hs == HS - 1
                    nc.tensor.matmul(psg, gt[:, hs, :], xt[:, hs, :],
                                     start=first, stop=last)
                    nc.tensor.matmul(psu, ut[:, hs, :], xt[:, hs, :],
                                     start=first, stop=last)
                sg = sgp.tile([P, T], F32, tag="sg")
                if use_silu:
                    # native HW silu: one ACT op frees psg immediately
                    nc.scalar.activation(sg, psg,
                                         mybir.ActivationFunctionType.Silu)
                else:
                    # CoreSim lacks Silu: sigmoid + extra DVE mul
                    nc.scalar.activation(sg, psg,
                                         mybir.ActivationFunctionType.Sigmoid)
                    nc.vector.tensor_mul(sg, sg, psg)
                nc.vector.tensor_mul(hm[:, isub, :], sg, psu)

            # ---- pass 2: down projection ----
            # The drain of o-chunk osc (PSUM->SBUF copy + y DMA) is issued
            # only after the first DRAIN_AT wd loads of chunk osc+1: the y
            # DMAs block (waiting on the copies) at the head of the FIFO DMA
            # queues, and anything issued behind them stalls, starving the PE
            # at every chunk boundary.
            DRAIN_AT = 24

            def drain(osc, pys):
                # alternate ACT/DVE so the four copies run pairwise-parallel
                for tt in range(TT):
                    yo = yop.tile([P, OCW], F32, tag="yo")
                    if tt % 2 == 0:
                        nc.scalar.activation(yo, pys[tt],
                                             mybir.ActivationFunctionType.Copy)
                    else:
                        nc.vector.tensor_scalar_mul(yo, pys[tt], 1.0)
                    nc.sync.dma_start(
                        out=y_d[tt, :, osc * OCW:(osc + 1) * OCW], in_=yo)

            pend = None
            for osc in range(NO):
                pys = [ps.tile([P, OCW], F32, tag="ps", name=f"py{tt}")
                       for tt in range(TT)]
                for isub in range(NI):
                    wdt = wdp.tile([P, OCW], BF16, tag="wd")
                    nc.sync.dma_start(out=wdt, in_=wd_d[osc, isub])
                    for tt in range(TT):
                        nc.tensor.matmul(
                            pys[tt],
                            hm[:, isub, tt * P:(tt + 1) * P],
                            wdt,
                            start=(isub == 0), stop=(isub == NI - 1),
                        )
                    if isub == DRAIN_AT and pend is not None:
                        drain(*pend)
                        pend = None
                pend = (osc, pys)
            drain(*pend)

    nc.compile()
    return nc


def prep_weights(Wg, Wu, Wd):
    """Host-side re-tiling of the weights into the device DMA layouts (bf16).

    Pads I up to a multiple of 128 with zeros (no-op for I=11008=86*128);
    padded hm columns are silu(0)*0 = 0 so they contribute nothing to y.
    """
    Iin, Hh = Wg.shape
    HS = Hh // P
    NI = -(-Iin // P)
    IPAD = NI * P
    NO = Hh // OCW

    if IPAD != Iin:
        Wg_p = np.zeros((IPAD, Hh), np.float32)
        Wg_p[:Iin] = Wg
        Wu_p = np.zeros((IPAD, Hh), np.float32)
        Wu_p[:Iin] = Wu
        Wd_p = np.zeros((Hh, IPAD), np.float32)
        Wd_p[:, :Iin] = Wd
    else:
        Wg_p, Wu_p, Wd_p = Wg, Wu, Wd

    # wg[isub, p, hs, ii] = Wg_p[isub*128 + ii, hs*128 + p]
    wg_host = np.ascontiguousarray(
        Wg_p.reshape(NI, P, HS, P).transpose(0, 3, 2, 1).astype(BF16_NP))
    wu_host = np.ascontiguousarray(
        Wu_p.reshape(NI, P, HS, P).transpose(0, 3, 2, 1).astype(BF16_NP))
    # wd[osc, isub, p, oo] = Wd_p[osc*512 + oo, isub*128 + p]
    wd_host = np.ascontiguousarray(
        Wd_p.reshape(NO, OCW, NI, P).transpose(0, 2, 3, 1).astype(BF16_NP))
    return wg_host, wu_host, wd_host


def prep_x_shard(x2, c, T):
    """x2 [tokens, H] -> core c's [HS, 128, T] bf16 tile layout."""
    Hh = x2.shape[1]
    xs = x2[c * T:(c + 1) * T]  # [T, H]
    return np.ascontiguousarray(
        xs.reshape(T, Hh // P, P).transpose(1, 2, 0).astype(BF16_NP))


def run_on_cores(nc, in_maps, **kwargs):
    return run_bass_kernel_spmd(nc, in_maps, core_ids=list(range(len(in_maps))), **kwargs)


_NC_CACHE = {}


def _get_nc():
    key = (T, H, I)
    if key not in _NC_CACHE:
        _NC_CACHE[key] = build_nc(T, H, I)
    return _NC_CACHE[key]


def kernel(x, Wg, Wu, Wd, _trace=False, _trace_kwargs=None):
    x = np.asarray(x, np.float32)
    Wg = np.asarray(Wg, np.float32)
    Wu = np.asarray(Wu, np.float32)
    Wd = np.asarray(Wd, np.float32)

    nc = _get_nc()
    wg_host, wu_host, wd_host = prep_weights(Wg, Wu, Wd)
    x2 = x.reshape(B * S, H)
    in_maps = [
        {
            "x": prep_x_shard(x2, c, T),
            "wg": wg_host,
            "wu": wu_host,
            "wd": wd_host,
        }
        for c in range(NCORES)
    ]
    kwargs = {}
    if _trace:
        kwargs["trace"] = True
        kwargs.update(_trace_kwargs or {})
    res = run_on_cores(nc, in_maps, **kwargs)
    shards = [res.results[c]["y"].reshape(T, H) for c in range(NCORES)]
    y = np.concatenate(shards, axis=0).reshape(B, S, H)
    if _trace:
        return y, res
    return y


# revision 18
# speedup vs baseline: 1.0050x; 1.0050x over previous
"""Fused SwiGLU MLP (gate/up/down) Trainium2 Bass kernel.

Problem: y = down( silu(x @ Wg^T) * (x @ Wu^T) ) with
  x  [B=2, S=2048, H=4096]  f32
  Wg [I=11008, H]           f32   (gate proj, [out,in])
  Wu [I=11008, H]           f32
  Wd [H, I]                 f32

Strategy: data-parallel over tokens across the 8 NeuronCores.
Each core gets T = 4096/8 = 512 tokens and the full (replicated) weights,
computing the entire MLP for its token shard.  No collectives; the host
just concatenates the 8 token shards.

Per-core work is 70.9 G MAC = 8256 matmuls of 128x128x512.  All matmul
operands are bf16 (1 PE cycle/row at 2.4 GHz, same rate as f32r, but half
the HBM traffic: ~270 MB/core vs 570 MB f32, so DMA stays well clear of
the PE roofline of ~1.76 ms).  f32 PSUM accumulation keeps the rel error
~4e-3.

Two-pass structure per core (v2; the v1 chunked structure interleaved
gate/up and down per i-chunk, costing PSUM-bank contention + a DVE
accumulate of y per chunk):
  pass 1: for each of 86 i-subtiles: psg/psu[128i, T] accumulate over
          32 h-subtiles; hm[:, isub, :] = silu(psg) * psu  (bf16, resident:
          full [128, 86, 512] = 86 KB/partition in SBUF)
  pass 2: for each o-chunk (8 x 512) hold 4 PSUM banks py[128t, 512o]
          (one per 128-token tile) and accumulate over all 86 i-subtiles;
          wd tile loaded once per (osc, isub), reused for 4 matmuls.
          Drain via ACT copy -> DMA out.  No DVE adds, no y memset.

All transposes/tiling are done on HOST in numpy so every device DMA is a
plain contiguous partition-major copy:
  x_host  [HS=32, 128, T]      x^T tiled: [hs, p, t] = x[t, hs*128+p]
  wg_host [NI=86, 128, HS, 128]  [isub, p, hs, ii] = Wg[isub*128+ii, hs*128+p]
  wu_host same
  wd_host [NO=8, NI, 128, 512]   [osc, isub, p, oo] = Wd[osc*512+oo, isub*128+p]
  y out   [TT=4, 128, H]       y[tt*128+p, o]  f32
"""

import numpy as np
import ml_dtypes

import concourse.bass as bass
import concourse.mybir as mybir
import concourse.tile as tile
from concourse import bacc
from concourse.bass_utils import run_bass_kernel_spmd

F32 = mybir.dt.float32
BF16 = mybir.dt.bfloat16
BF16_NP = ml_dtypes.bfloat16
P = 128
OCW = 512  # o-chunk width (one PSUM bank of f32)

# full-size problem constants
B, S, H, I = 2, 2048, 4096, 11008
NCORES = 8
T = (B * S) // NCORES  # 512 tokens per core


def build_nc(T, H, I, wg_bufs=3, wd_bufs=8, use_silu=True):
    assert T % P == 0 and T <= 512
    assert H % OCW == 0 and I % P == 0
    HS = H // P   # h subtiles (contraction for gate/up)
    NI = I // P   # i subtiles (with I padded to a multiple of 128 by host)
    NO = H // OCW
    TT = T // P

    assert NI % 2 == 0 and HS % 4 == 0
    XC = HS // 4  # x DMA chunk: 4 h-subtiles -> 8 KB per partition line

    nc = bacc.Bacc("TRN2", target_bir_lowering=False, debug=False)
    x_d = nc.dram_tensor("x", [4, P, XC, T], BF16, kind="ExternalInput").ap()
    wg_d = nc.dram_tensor("wg", [NI, P, HS, P], BF16, kind="ExternalInput").ap()
    wu_d = nc.dram_tensor("wu", [NI, P, HS, P], BF16, kind="ExternalInput").ap()
    # wd pairs two i-subtiles per tile for 2 KB partition lines
    wd_d = nc.dram_tensor("wd", [NO, NI // 2, P, 2, OCW], BF16, kind="ExternalInput").ap()
    y_d = nc.dram_tensor("y", [TT, P, H], BF16, kind="ExternalOutput").ap()

    with tile.TileContext(nc) as tc:
        with (
            tc.tile_pool(name="xp", bufs=1) as xp,
            tc.tile_pool(name="hmp", bufs=1) as hmp,
            tc.tile_pool(name="wgp", bufs=wg_bufs) as wgp,
            tc.tile_pool(name="wup", bufs=wg_bufs) as wup,
            tc.tile_pool(name="wdp", bufs=wd_bufs) as wdp,
            tc.tile_pool(name="sgp", bufs=2) as sgp,
            tc.tile_pool(name="yop", bufs=4) as yop,
            tc.tile_pool(name="ps", bufs=8, space="PSUM") as ps,
        ):
            # DMA issue order matters: queues are FIFO, so stream the first
            # i-subtile's weights and x in exact consumption order (per
            # 8-h-subtile octet), then subtile 1's weights, so the PE starts
            # after ~0.4 MiB instead of waiting behind 4 MiB of x.  Tile
            # tracks slice-level deps, so partial-tile DMAs unblock the
            # matmuls that only read those slices.
            xt = xp.tile([P, HS, T], BF16, name="xt")
            gt0 = wgp.tile([P, HS, P], BF16, tag="wg", name="gt0")
            ut0 = wup.tile([P, HS, P], BF16, tag="wu", name="ut0")
            for c in range(4):
                lo = c * XC
                nc.sync.dma_start(out=gt0[:, lo:lo + XC, :], in_=wg_d[0, :, lo:lo + XC, :])
                nc.sync.dma_start(out=ut0[:, lo:lo + XC, :], in_=wu_d[0, :, lo:lo + XC, :])
                nc.sync.dma_start(out=xt[:, lo:lo + XC, :], in_=x_d[c])
            gt1 = wgp.tile([P, HS, P], BF16, tag="wg", name="gt1")
            nc.sync.dma_start(out=gt1, in_=wg_d[1])
            ut1 = wup.tile([P, HS, P], BF16, tag="wu", name="ut1")
            nc.sync.dma_start(out=ut1, in_=wu_d[1])
            head = [(gt0, ut0), (gt1, ut1)]
            # resident h_mid, [128i, isub, t] bf16 (86 KB/partition)
            hm = hmp.tile([P, NI, T], BF16)

            # ---- pass 1: gate/up projections + silu*up ----
            for isub in range(NI):
                if isub < 2:
                    gt, ut = head[isub]
                else:
                    gt = wgp.tile([P, HS, P], BF16, tag="wg")
                    nc.sync.dma_start(out=gt, in_=wg_d[isub])
                    ut = wup.tile([P, HS, P], BF16, tag="wu")
                    nc.sync.dma_start(out=ut, in_=wu_d[isub])
                psg = ps.tile([P, T], F32, tag="ps", name="psg")
                psu = ps.tile([P, T], F32, tag="ps", name="psu")
                for hs in range(HS):
                    first, last = hs == 0, hs == HS - 1
                    nc.tensor.matmul(psg, gt[:, hs, :], xt[:, hs, :],
                                     start=first, stop=last)
                    nc.tensor.matmul(psu, ut[:, hs, :], xt[:, hs, :],
                                     start=first, stop=last)
                sg = sgp.tile([P, T], F32, tag="sg")
                if use_silu:
                    # native HW silu: one ACT op frees psg immediately
                    nc.scalar.activation(sg, psg,
                                         mybir.ActivationFunctionType.Silu)
                else:
                    # CoreSim lacks Silu: sigmoid + extra DVE mul
                    nc.scalar.activation(sg, psg,
                                         mybir.ActivationFunctionType.Sigmoid)
                    nc.vector.tensor_mul(sg, sg, psg)
                nc.vector.tensor_mul(hm[:, isub, :], sg, psu)

            # ---- pass 2: down projection ----
            # The drain of o-chunk osc (PSUM->SBUF copy + y DMA) is issued
            # only after the first DRAIN_AT wd loads of chunk osc+1: the y
            # DMAs block (waiting on the copies) at the head of the FIFO DMA
            # queues, and anything issued behind them stalls, starving the PE
            # at every chunk boundary.
            DRAIN_AT = 12  # in i-subtile pairs

            def drain(osc, pys):
                # alternate ACT/DVE so the four copies run pairwise-parallel
                for tt in range(TT):
                    yo = yop.tile([P, OCW], BF16, tag="yo")
                    if tt % 2 == 0:
                        nc.scalar.activation(yo, pys[tt],
                                             mybir.ActivationFunctionType.Copy)
                    else:
                        nc.vector.tensor_scalar_mul(yo, pys[tt], 1.0)
                    nc.sync.dma_start(
                        out=y_d[tt, :, osc * OCW:(osc + 1) * OCW], in_=yo)

            pend = None
            for osc in range(NO):
                pys = [ps.tile([P, OCW], F32, tag="ps", name=f"py{tt}")
                       for tt in range(TT)]
                for ip in range(NI // 2):
                    wdt = wdp.tile([P, 2, OCW], BF16, tag="wd")
                    nc.sync.dma_start(out=wdt, in_=wd_d[osc, ip])
                    for k in range(2):
                        isub = 2 * ip + k
                        for tt in range(TT):
                            nc.tensor.matmul(
                                pys[tt],
                                hm[:, isub, tt * P:(tt + 1) * P],
                                wdt[:, k, :],
                                start=(isub == 0), stop=(isub == NI - 1),
                            )
                    if ip == DRAIN_AT and pend is not None:
                        drain(*pend)
                        pend = None
                pend = (osc, pys)
            drain(*pend)

    nc.compile()
    return nc


def prep_weights(Wg, Wu, Wd):
    """Host-side re-tiling of the weights into the device DMA layouts (bf16).

    Pads I up to a multiple of 128 with zeros (no-op for I=11008=86*128);
    padded hm columns are silu(0)*0 = 0 so they contribute nothing to y.
    """
    Iin, Hh = Wg.shape
    HS = Hh // P
    NI = -(-Iin // P)
    IPAD = NI * P
    NO = Hh // OCW

    if IPAD != Iin:
        Wg_p = np.zeros((IPAD, Hh), np.float32)
        Wg_p[:Iin] = Wg
        Wu_p = np.zeros((IPAD, Hh), np.float32)
        Wu_p[:Iin] = Wu
        Wd_p = np.zeros((Hh, IPAD), np.float32)
        Wd_p[:, :Iin] = Wd
    else:
        Wg_p, Wu_p, Wd_p = Wg, Wu, Wd

    # wg[isub, p, hs, ii] = Wg_p[isub*128 + ii, hs*128 + p]
    wg_host = np.ascontiguousarray(
        Wg_p.reshape(NI, P, HS, P).transpose(0, 3, 2, 1).astype(BF16_NP))
    wu_host = np.ascontiguousarray(
        Wu_p.reshape(NI, P, HS, P).transpose(0, 3, 2, 1).astype(BF16_NP))
    # wd[osc, ip, p, k, oo] = Wd_p[osc*512 + oo, (2*ip+k)*128 + p]
    wd_host = np.ascontiguousarray(
        Wd_p.reshape(NO, OCW, NI // 2, 2, P).transpose(0, 2, 4, 3, 1).astype(BF16_NP))
    return wg_host, wu_host, wd_host


def prep_x_shard(x2, c, T):
    """x2 [tokens, H] -> core c's [4, 128, HS/4, T] bf16 chunk layout."""
    Hh = x2.shape[1]
    XC = Hh // P // 4
    xs = x2[c * T:(c + 1) * T]  # [T, H]
    # [chunk, p, hs_in_chunk, t] = xs[t, ((chunk*XC + hs)*128 + p)]
    return np.ascontiguousarray(
        xs.reshape(T, 4, XC, P).transpose(1, 3, 2, 0).astype(BF16_NP))


def run_on_cores(nc, in_maps, **kwargs):
    return run_bass_kernel_spmd(nc, in_maps, core_ids=list(range(len(in_maps))), **kwargs)


_NC_CACHE = {}


def _get_nc():
    key = (T, H, I)
    if key not in _NC_CACHE:
        _NC_CACHE[key] = build_nc(T, H, I)
    return _NC_CACHE[key]


def kernel(x, Wg, Wu, Wd, _trace=False, _trace_kwargs=None):
    x = np.asarray(x, np.float32)
    Wg = np.asarray(Wg, np.float32)
    Wu = np.asarray(Wu, np.float32)
    Wd = np.asarray(Wd, np.float32)

    nc = _get_nc()
    wg_host, wu_host, wd_host = prep_weights(Wg, Wu, Wd)
    x2 = x.reshape(B * S, H)
    in_maps = [
        {
            "x": prep_x_shard(x2, c, T),
            "wg": wg_host,
            "wu": wu_host,
            "wd": wd_host,
        }
        for c in range(NCORES)
    ]
    kwargs = {}
    if _trace:
        kwargs["trace"] = True
        kwargs.update(_trace_kwargs or {})
    res = run_on_cores(nc, in_maps, **kwargs)
    shards = [np.asarray(res.results[c]["y"]).astype(np.float32).reshape(T, H)
              for c in range(NCORES)]
    y = np.concatenate(shards, axis=0).reshape(B, S, H)
    if _trace:
        return y, res
    return y
